# revision 7
# baseline (speedup 1.0000x reference)
"""DCL (decoupled contrastive learning) loss on Trainium2, 8 NeuronCores.

Math (reference):
    img_n = l2norm(img);  mol_n = l2norm(mol)   [N, C], N=8192, C=128
    sim00 = img_n @ img_n.T / T ; sim01 = img_n @ mol_n.T / T ; sim11 = mol_n @ mol_n.T / T
    loss = mean(-diag(sim01))
         + 0.5 * mean( lse_row(sim00\\diag) + lse_row(sim01\\diag)
                     + lse_col(sim01\\diag) + lse_row(sim11\\diag) )

Device strategy (row-data-parallel over 8 cores):
  Each core gets the full img/mol (for the all-gathered side) plus its own
  N/8 row slice. On device: l2-normalize (fp32 stats), cast bf16, transpose
  via PE so both operands have C on partitions; compute its [1024 x 8192]
  blocks of the three gram matrices on PE; exp(10*x) fused with row-sum
  accumulation on the scalar engine straight out of PSUM; column-sums of
  exp(sim01) via ones-matmuls (these give the sim10 row-sums).  Device
  outputs are O(N) partial sums; the host adds partials across cores,
  subtracts exp(diag), takes log and means (O(N) scalar work).
"""

import numpy as np

import concourse.bass as bass
import concourse.tile as tile
from concourse import bacc, mybir
from concourse.bass_utils import run_bass_kernel_spmd
from concourse.masks import make_identity

F32 = mybir.dt.float32
BF16 = mybir.dt.bfloat16
AF = mybir.ActivationFunctionType
ALU = mybir.AluOpType

N_TOTAL = 8192
C = 128
N_CORES = 8
INV_T = 10.0  # 1 / TEMPERATURE


def _chunks(width, chunk):
    out = []
    s = 0
    while s < width:
        w = min(chunk, width - s)
        out.append((s, w))
        s += w
    return out


def build(n_total=N_TOTAL, n_cores=N_CORES, chunk=1536, sub_bank_transpose=True):
    """Build the per-core Tile program (SPMD: same program, different data)."""
    P = 128
    nm = n_total // n_cores          # rows owned by this core
    t_all = n_total // P             # 128-row tiles in the full matrices
    t_mine = nm // P                 # 128-row tiles in my slice
    n_chunks = len(_chunks(n_total, chunk))

    nc = bacc.Bacc("TRN2", target_bir_lowering=False, debug=False,
                   num_devices=n_cores)

    d_img = nc.dram_tensor("img", [n_total, C], F32, kind="ExternalInput").ap()
    d_mol = nc.dram_tensor("mol", [n_total, C], F32, kind="ExternalInput").ap()
    d_img_m = nc.dram_tensor("img_mine", [nm, C], F32, kind="ExternalInput").ap()
    d_mol_m = nc.dram_tensor("mol_mine", [nm, C], F32, kind="ExternalInput").ap()

    # rowsums: per (matrix, mtile, chunk) partial row sums of exp(10*cos)
    d_rowsums = nc.dram_tensor("rowsums", [P, 3 * t_mine * n_chunks], F32,
                               kind="ExternalOutput").ap()
    # colsums: this core's partial column sums of exp(10*sim01)
    d_colsums = nc.dram_tensor("colsums", [1, n_total], F32,
                               kind="ExternalOutput").ap()
    # diags: raw cosine self/cross diagonals for my rows: rows 0:xx 1:xy 2:yy
    d_diags = nc.dram_tensor("diags", [3, nm], F32, kind="ExternalOutput").ap()

    with tile.TileContext(nc) as tc:
        with (
            tc.tile_pool(name="big", bufs=1) as big,
            tc.tile_pool(name="work", bufs=3) as work,
            tc.tile_pool(name="expb", bufs=t_mine + 2) as expb,
            tc.tile_pool(name="sim", bufs=2, space="PSUM") as simp,
            tc.tile_pool(name="misc", bufs=2, space="PSUM") as miscp,
        ):
            # ---------------- constants / big buffers ----------------
            ident = big.tile([P, P], BF16, tag="ident")
            make_identity(nc, ident)
            ones_b = big.tile([P, 1], BF16, tag="ones")
            nc.vector.memset(ones_b, 1.0)
            ones_f = big.tile([P, 1], F32, tag="ones_f")
            nc.vector.memset(ones_f, 1.0)

            # raw fp32 inputs resident in SBUF, tile-major: [P, t, C]
            raws = {}
            rawm = {}
            xT = {}   # normalized, transposed, bf16: [P(C), n_total]
            xTm = {}  # normalized, transposed, bf16: [P(C), nm]
            for name, src, srcm, tiles, tiles_m in (
                ("x", d_img, d_img_m, t_all, t_mine),
                ("y", d_mol, d_mol_m, t_all, t_mine),
            ):
                raws[name] = big.tile([P, t_all, C], F32, tag=f"raw_{name}", name=f"raw_{name}")
                rawm[name] = big.tile([P, t_mine, C], F32, tag=f"rawm_{name}", name=f"rawm_{name}")
                xT[name] = big.tile([P, n_total], BF16, tag=f"T_{name}", name=f"T_{name}")
                xTm[name] = big.tile([P, nm], BF16, tag=f"Tm_{name}", name=f"Tm_{name}")

            # per-row sumsq then rsqrt, all tiles side by side
            # columns: [x tiles | y tiles | xm tiles | ym tiles]
            ncols = 2 * t_all + 2 * t_mine
            ss = big.tile([P, ncols], F32, tag="ss")
            rs = big.tile([P, ncols], F32, tag="rs")

            # rowsums accumulate target
            rows_sb = big.tile([P, 3 * t_mine * n_chunks], F32, tag="rows_sb")

            # ---------------- phase A: load, stats ----------------
            specs = [("x", raws["x"], d_img, t_all, 0),
                     ("y", raws["y"], d_mol, t_all, t_all),
                     ("xm", rawm["x"], d_img_m, t_mine, 2 * t_all),
                     ("ym", rawm["y"], d_mol_m, t_mine, 2 * t_all + t_mine)]
            SLAB = 8  # tiles per wide DVE op
            for _, dst, src, tiles, _c0 in specs:
                src3 = src.rearrange("(t p) c -> p t c", p=P)
                for s in range(0, tiles, SLAB):
                    se = min(s + SLAB, tiles)
                    nc.sync.dma_start(out=dst[:, s:se, :], in_=src3[:, s:se, :])

            for _, dst, src, tiles, c0 in specs:
                for s in range(0, tiles, SLAB):
                    se = min(s + SLAB, tiles)
                    sq = work.tile([P, SLAB, C], F32, tag="sq")
                    nc.vector.tensor_mul(sq[:, : se - s, :], dst[:, s:se, :],
                                         dst[:, s:se, :])
                    nc.vector.reduce_sum(out=ss[:, c0 + s:c0 + se],
                                         in_=sq[:, : se - s, :],
                                         axis=mybir.AxisListType.X)
            # norm = sqrt(ss); rs = 1/norm   (fp32, full width at once)
            norm = work.tile([P, ncols], F32, tag="norm")
            nc.scalar.activation(out=norm, in_=ss, func=AF.Sqrt)
            nc.vector.reciprocal(out=rs, in_=norm)

            # ---------------- phase A: normalize + transpose ----------------
            def norm_transpose(raw, c0, tiles, dstT):
                # raw: [P, tiles, C] fp32; dstT: [P(C), tiles*128] bf16
                for t in range(tiles):
                    nb = work.tile([P, C], BF16, tag="normtile")
                    nc.vector.tensor_scalar_mul(nb, raw[:, t, :],
                                                rs[:, c0 + t:c0 + t + 1])
                    if sub_bank_transpose:
                        grp = t % 4
                        if grp == 0:
                            pt = miscp.tile([P, 4 * P], BF16, tag="misc")
                        nc.tensor.transpose(pt[:, grp * P:(grp + 1) * P], nb, ident)
                        if grp == 3 or t == tiles - 1:
                            w = (grp + 1) * P
                            nc.vector.tensor_copy(
                                out=dstT[:, (t - grp) * P:(t - grp) * P + w],
                                in_=pt[:, :w])
                    else:
                        pt = miscp.tile([P, P], BF16, tag="misc")
                        nc.tensor.transpose(pt, nb, ident)
                        nc.vector.tensor_copy(out=dstT[:, t * P:(t + 1) * P], in_=pt)

            norm_transpose(raws["x"], 0, t_all, xT["x"])
            norm_transpose(raws["y"], t_all, t_all, xT["y"])
            norm_transpose(rawm["x"], 2 * t_all, t_mine, xTm["x"])
            norm_transpose(rawm["y"], 2 * t_all + t_mine, t_mine, xTm["y"])

            # ---------------- phase B: diagonals of my rows ----------------
            # diag[i] = sum_c a_hat[c,i] * b_hat[c,i]  -> ones-matmul col reduce
            for row, (a, b) in enumerate((("x", "x"), ("x", "y"), ("y", "y"))):
                prod = work.tile([P, nm], F32, tag="diagprod")
                nc.vector.tensor_mul(prod, xTm[a], xTm[b])
                for s in range(0, nm, 512):
                    w = min(512, nm - s)
                    dp = miscp.tile([1, 512], F32, tag="misc")
                    nc.tensor.matmul(dp[:, :w], ones_f, prod[:, s:s + w],
                                     start=True, stop=True)
                    dsb = work.tile([1, 512], F32, tag="dsb")
                    nc.vector.tensor_copy(out=dsb[:, :w], in_=dp[:, :w])
                    nc.sync.dma_start(out=d_diags[row:row + 1, s:s + w],
                                      in_=dsb[:, :w])

            # ---------------- phase C: gram blocks, exp, row/col sums ------
            # chunk-outer, mtile-inner; sim01 column sums accumulate in PSUM
            # across the 8 mtiles (matmul accumulation), one copy per chunk.
            mats = ((0, "x", "x"), (1, "x", "y"), (2, "y", "y"))
            chunk_list = _chunks(n_total, chunk)
            for mat, a, b in mats:
                for ci, (cs, cw) in enumerate(chunk_list):
                    ebs = []
                    for m in range(t_mine):
                        lhsT = xTm[a][:, m * P:(m + 1) * P]
                        ps = simp.tile([P, chunk], F32, tag="sim")
                        for s in range(0, cw, 512):
                            nc.tensor.matmul(ps[:, s:s + 512], lhsT,
                                             xT[b][:, cs + s:cs + s + 512],
                                             start=True, stop=True)
                        eb = expb.tile([P, chunk], BF16, tag="eb",
                                       name=f"eb_{mat}_{ci}_{m}")
                        col = (mat * t_mine + m) * n_chunks + ci
                        nc.scalar.activation(out=eb[:, :cw], in_=ps[:, :cw],
                                             func=AF.Exp, scale=INV_T,
                                             accum_out=rows_sb[:, col:col + 1])
                        ebs.append(eb)
                    if mat == 1:
                        for s in range(0, cw, 512):
                            cp = miscp.tile([1, 512], F32, tag="misc")
                            for m in range(t_mine):
                                nc.tensor.matmul(cp, ones_b,
                                                 ebs[m][:, s:s + 512],
                                                 start=(m == 0),
                                                 stop=(m == t_mine - 1))
                            csb = work.tile([1, 512], F32, tag="dsb")
                            nc.vector.tensor_copy(out=csb, in_=cp)
                            nc.sync.dma_start(
                                out=d_colsums[0:1, cs + s:cs + s + 512],
                                in_=csb)

            nc.sync.dma_start(out=d_rowsums, in_=rows_sb)

    nc.finalize()
    return nc


_NC_CACHE = {}


def _get_nc(n_total, n_cores, chunk=1536):
    key = (n_total, n_cores, chunk)
    if key not in _NC_CACHE:
        _NC_CACHE[key] = build(n_total, n_cores, chunk)
    return _NC_CACHE[key]


def _run(img, mol, trace=False, n_cores=N_CORES):
    img = np.ascontiguousarray(np.asarray(img, dtype=np.float32))
    mol = np.ascontiguousarray(np.asarray(mol, dtype=np.float32))
    n_total = img.shape[0]
    nm = n_total // n_cores
    nc = _get_nc(n_total, n_cores)

    in_maps = [
        {
            "img": img,
            "mol": mol,
            "img_mine": img[r * nm:(r + 1) * nm],
            "mol_mine": mol[r * nm:(r + 1) * nm],
        }
        for r in range(n_cores)
    ]
    res = run_bass_kernel_spmd(nc, in_maps, list(range(n_cores)), trace=trace)

    # ---------------- host combine: O(N) work ----------------
    P = 128
    t_mine = nm // P
    n_chunks = len(_chunks(n_total, 1536))
    rowsum = np.zeros((3, n_total), dtype=np.float64)
    colsum01 = np.zeros(n_total, dtype=np.float64)
    diags = np.zeros((3, n_total), dtype=np.float64)
    for r in range(n_cores):
        out = res.results[r]
        rw = out["rowsums"].astype(np.float64)      # [128, 3*t_mine*n_chunks]
        cs = out["colsums"].astype(np.float64)      # [1, n_total]
        dg = out["diags"].astype(np.float64)        # [3, nm]
        for mat in range(3):
            for m in range(t_mine):
                cols = slice((mat * t_mine + m) * n_chunks,
                             (mat * t_mine + m + 1) * n_chunks)
                rows = slice(r * nm + m * P, r * nm + (m + 1) * P)
                rowsum[mat, rows] = rw[:, cols].sum(axis=1)
        colsum01 += cs.sum(axis=0)
        diags[:, r * nm:(r + 1) * nm] = dg

    ed = np.exp(INV_T * diags)  # exp(10 * raw cosine diag)
    s00 = rowsum[0] - ed[0]
    s01r = rowsum[1] - ed[1]
    s01c = colsum01 - ed[1]
    s11 = rowsum[2] - ed[2]
    pos = -INV_T * diags[1]
    loss = pos.mean() + 0.5 * (np.log(s00) + np.log(s01r)
                               + np.log(s01c) + np.log(s11)).mean()
    return np.array(loss, dtype=np.float32), res


def kernel(img_rep, mol_rep):
    loss, _ = _run(img_rep, mol_rep)
    return loss


# revision 8
# speedup vs baseline: 1.0953x; 1.0953x over previous
"""DCL (decoupled contrastive learning) loss on Trainium2, 8 NeuronCores.

Math (reference):
    img_n = l2norm(img);  mol_n = l2norm(mol)   [N, C], N=8192, C=128
    sim00 = img_n @ img_n.T / T ; sim01 = img_n @ mol_n.T / T ; sim11 = mol_n @ mol_n.T / T
    loss = mean(-diag(sim01))
         + 0.5 * mean( lse_row(sim00\\diag) + lse_row(sim01\\diag)
                     + lse_col(sim01\\diag) + lse_row(sim11\\diag) )

Device strategy (row-data-parallel over 8 cores):
  Each core gets the full img/mol (the "all-gathered" side) plus its own
  N/8 row slice.  On device: l2-normalize (fp32 stats), cast bf16,
  transpose via PE so both matmul operands have C on partitions; compute
  its [N/8 x N] blocks of the three gram matrices on PE; exp(10*x) fused
  with row-sum accumulation on the scalar engine straight out of PSUM;
  column-sums of exp(sim01) via ones-matmuls accumulated in PSUM across
  the row tiles (these give the sim10 row-sums).  Device outputs are
  O(N) partial sums; the host adds partials across cores, subtracts
  exp(diag), takes log and means (O(N) scalar work).

  Emission order is pipelined: img's normalize/transpose, then all of
  sim00 (whose long exp stream hides mol's normalize/transpose), then
  sim01, then sim11.
"""

import numpy as np

import concourse.bass as bass
import concourse.tile as tile
from concourse import bacc, mybir
from concourse.bass_utils import run_bass_kernel_spmd
from concourse.masks import make_identity

F32 = mybir.dt.float32
BF16 = mybir.dt.bfloat16
AF = mybir.ActivationFunctionType
ALU = mybir.AluOpType

N_TOTAL = 8192
C = 128
N_CORES = 8
INV_T = 10.0  # 1 / TEMPERATURE


def _chunks(width, chunk):
    out = []
    s = 0
    while s < width:
        w = min(chunk, width - s)
        out.append((s, w))
        s += w
    return out


def build(n_total=N_TOTAL, n_cores=N_CORES, chunk=1536):
    """Build the per-core Tile program (SPMD: same program, different data)."""
    P = 128
    nm = n_total // n_cores          # rows owned by this core
    t_all = n_total // P             # 128-row tiles in the full matrices
    t_mine = nm // P                 # 128-row tiles in my slice
    chunk_list = _chunks(n_total, chunk)
    n_chunks = len(chunk_list)

    nc = bacc.Bacc("TRN2", target_bir_lowering=False, debug=False,
                   num_devices=n_cores)

    d_img = nc.dram_tensor("img", [n_total, C], F32, kind="ExternalInput").ap()
    d_mol = nc.dram_tensor("mol", [n_total, C], F32, kind="ExternalInput").ap()
    d_img_m = nc.dram_tensor("img_mine", [nm, C], F32, kind="ExternalInput").ap()
    d_mol_m = nc.dram_tensor("mol_mine", [nm, C], F32, kind="ExternalInput").ap()

    d_rowsums = nc.dram_tensor("rowsums", [P, 3 * t_mine * n_chunks], F32,
                               kind="ExternalOutput").ap()
    d_colsums = nc.dram_tensor("colsums", [1, n_total], F32,
                               kind="ExternalOutput").ap()
    d_diags = nc.dram_tensor("diags", [3, nm], F32, kind="ExternalOutput").ap()

    with tile.TileContext(nc) as tc:
        with (
            tc.tile_pool(name="big", bufs=1) as big,
            tc.tile_pool(name="work", bufs=3) as work,
            tc.tile_pool(name="expb", bufs=t_mine + 2) as expb,
            tc.tile_pool(name="sim", bufs=2, space="PSUM") as simp,
            tc.tile_pool(name="misc", bufs=2, space="PSUM") as miscp,
        ):
            ident = big.tile([P, P], BF16, tag="ident")
            make_identity(nc, ident)
            ones_b = big.tile([P, 1], BF16, tag="ones")
            nc.vector.memset(ones_b, 1.0)
            ones_f = big.tile([P, 1], F32, tag="ones_f")
            nc.vector.memset(ones_f, 1.0)

            specs = {
                "x": (d_img, t_all), "y": (d_mol, t_all),
                "xm": (d_img_m, t_mine), "ym": (d_mol_m, t_mine),
            }
            T = {}
            for k, (_, tiles) in specs.items():
                T[k] = big.tile([P, tiles * P], BF16, tag=f"T_{k}",
                                name=f"T_{k}")

            rows_sb = big.tile([P, 3 * t_mine * n_chunks], F32, tag="rows_sb")

            SLAB = 8

            def phase_a(key):
                """load -> sumsq -> rsqrt -> normalize(bf16) -> transpose -> T[key]"""
                src, tiles = specs[key]
                raw = big.tile([P, tiles, C], F32, tag=f"raw_{key}",
                               name=f"raw_{key}")
                ss = big.tile([P, tiles], F32, tag=f"ss_{key}", name=f"ss_{key}")
                rs = big.tile([P, tiles], F32, tag=f"rs_{key}", name=f"rs_{key}")
                src3 = src.rearrange("(t p) c -> p t c", p=P)
                for s in range(0, tiles, SLAB):
                    se = min(s + SLAB, tiles)
                    nc.sync.dma_start(out=raw[:, s:se, :], in_=src3[:, s:se, :])
                for s in range(0, tiles, SLAB):
                    se = min(s + SLAB, tiles)
                    sq = work.tile([P, SLAB, C], F32, tag="sq")
                    nc.vector.tensor_mul(sq[:, : se - s, :], raw[:, s:se, :],
                                         raw[:, s:se, :])
                    nc.vector.reduce_sum(out=ss[:, s:se], in_=sq[:, : se - s, :],
                                         axis=mybir.AxisListType.X)
                nrm_f = work.tile([P, tiles], F32, tag="nrm_f")
                nc.scalar.activation(out=nrm_f, in_=ss, func=AF.Sqrt)
                nc.vector.reciprocal(out=rs, in_=nrm_f)
                # normalize a slab at a time (rs broadcast along C, step-0 AP)
                cgrp = 0
                for s in range(0, tiles, SLAB):
                    se = min(s + SLAB, tiles)
                    nrm = work.tile([P, SLAB, C], BF16, tag="nrm")
                    rs_sl = rs[:, s:se]
                    rs_b = bass.AP(tensor=rs_sl.tensor, offset=rs_sl.offset,
                                   ap=[rs_sl.ap[0], rs_sl.ap[1], [0, C]])
                    nc.vector.tensor_mul(nrm[:, : se - s, :], raw[:, s:se, :],
                                         rs_b)
                    for t in range(s, se):
                        grp = t % 4
                        if grp == 0:
                            pt = miscp.tile([P, 4 * P], BF16, tag="misc",
                                            name=f"pt_{key}_{t}")
                        nc.tensor.transpose(pt[:, grp * P:(grp + 1) * P],
                                            nrm[:, t - s, :], ident)
                        if grp == 3 or t == tiles - 1:
                            w = (grp + 1) * P
                            dst = T[key][:, (t - grp) * P:(t - grp) * P + w]
                            # alternate copies between DVE and ACT
                            if cgrp % 2 == 0:
                                nc.vector.tensor_copy(out=dst, in_=pt[:, :w])
                            else:
                                nc.scalar.copy(out=dst, in_=pt[:, :w])
                            cgrp += 1

            def diag_block():
                # diag[i] = sum_c a_hat[c,i] * b_hat[c,i] -> ones-matmul
                for row, (a, b) in enumerate((("xm", "xm"), ("xm", "ym"),
                                              ("ym", "ym"))):
                    prod = work.tile([P, nm], F32, tag="diagprod", bufs=2)
                    nc.vector.tensor_mul(prod, T[a], T[b])
                    for s in range(0, nm, 512):
                        w = min(512, nm - s)
                        dp = miscp.tile([1, 512], F32, tag="misc")
                        nc.tensor.matmul(dp[:, :w], ones_f, prod[:, s:s + w],
                                         start=True, stop=True)
                        dsb = work.tile([1, 512], F32, tag="dsb")
                        nc.vector.tensor_copy(out=dsb[:, :w], in_=dp[:, :w])
                        nc.sync.dma_start(out=d_diags[row:row + 1, s:s + w],
                                          in_=dsb[:, :w])

            def gram_block(mat, a, b):
                """rows of T[a] x all of T[b]: exp rowsums (+colsums if mat==1)"""
                for ci, (cs, cw) in enumerate(chunk_list):
                    ebs = []
                    for m in range(t_mine):
                        lhsT = T[a][:, m * P:(m + 1) * P]
                        ps = simp.tile([P, chunk], F32, tag="sim")
                        for s in range(0, cw, 512):
                            nc.tensor.matmul(ps[:, s:s + 512], lhsT,
                                             T[b][:, cs + s:cs + s + 512],
                                             start=True, stop=True)
                        eb = expb.tile([P, chunk], BF16, tag="eb",
                                       name=f"eb_{mat}_{ci}_{m}")
                        col = (mat * t_mine + m) * n_chunks + ci
                        nc.scalar.activation(out=eb[:, :cw], in_=ps[:, :cw],
                                             func=AF.Exp, scale=INV_T,
                                             accum_out=rows_sb[:, col:col + 1])
                        ebs.append(eb)
                    if mat == 1:
                        for s in range(0, cw, 512):
                            cp = miscp.tile([1, 512], F32, tag="misc")
                            for m in range(t_mine):
                                nc.tensor.matmul(cp, ones_b,
                                                 ebs[m][:, s:s + 512],
                                                 start=(m == 0),
                                                 stop=(m == t_mine - 1))
                            csb = work.tile([1, 512], F32, tag="dsb")
                            nc.vector.tensor_copy(out=csb, in_=cp)
                            nc.sync.dma_start(
                                out=d_colsums[0:1, cs + s:cs + s + 512],
                                in_=csb)

            # ---- pipelined emission order ----
            phase_a("x")
            phase_a("xm")
            gram_block(0, "xm", "x")     # mol's transposes hide under this
            phase_a("y")
            phase_a("ym")
            diag_block()
            gram_block(1, "xm", "y")
            gram_block(2, "ym", "y")

            nc.sync.dma_start(out=d_rowsums, in_=rows_sb)

    nc.finalize()
    return nc


_NC_CACHE = {}


def _get_nc(n_total, n_cores, chunk=1536):
    key = (n_total, n_cores, chunk)
    if key not in _NC_CACHE:
        _NC_CACHE[key] = build(n_total, n_cores, chunk)
    return _NC_CACHE[key]


def _run(img, mol, trace=False, n_cores=N_CORES):
    img = np.ascontiguousarray(np.asarray(img, dtype=np.float32))
    mol = np.ascontiguousarray(np.asarray(mol, dtype=np.float32))
    n_total = img.shape[0]
    nm = n_total // n_cores
    nc = _get_nc(n_total, n_cores)

    in_maps = [
        {
            "img": img,
            "mol": mol,
            "img_mine": img[r * nm:(r + 1) * nm],
            "mol_mine": mol[r * nm:(r + 1) * nm],
        }
        for r in range(n_cores)
    ]
    res = run_bass_kernel_spmd(nc, in_maps, list(range(n_cores)), trace=trace)

    # ---------------- host combine: O(N) work ----------------
    P = 128
    t_mine = nm // P
    n_chunks = len(_chunks(n_total, 1536))
    rowsum = np.zeros((3, n_total), dtype=np.float64)
    colsum01 = np.zeros(n_total, dtype=np.float64)
    diags = np.zeros((3, n_total), dtype=np.float64)
    for r in range(n_cores):
        out = res.results[r]
        rw = out["rowsums"].astype(np.float64)      # [128, 3*t_mine*n_chunks]
        for mat in range(3):
            for m in range(t_mine):
                cols = slice((mat * t_mine + m) * n_chunks,
                             (mat * t_mine + m + 1) * n_chunks)
                rows = slice(r * nm + m * P, r * nm + (m + 1) * P)
                rowsum[mat, rows] = rw[:, cols].sum(axis=1)
        colsum01 += out["colsums"].astype(np.float64)[0]
        diags[:, r * nm:(r + 1) * nm] = out["diags"].astype(np.float64)

    ed = np.exp(INV_T * diags)  # exp(10 * raw cosine diag)
    s00 = rowsum[0] - ed[0]
    s01r = rowsum[1] - ed[1]
    s01c = colsum01 - ed[1]
    s11 = rowsum[2] - ed[2]
    pos = -INV_T * diags[1]
    loss = pos.mean() + 0.5 * (np.log(s00) + np.log(s01r)
                               + np.log(s01c) + np.log(s11)).mean()
    return np.array(loss, dtype=np.float32), res


def kernel(img_rep, mol_rep):
    loss, _ = _run(img_rep, mol_rep)
    return loss


# revision 9
# speedup vs baseline: 1.2966x; 1.1837x over previous
"""DCL (decoupled contrastive learning) loss on Trainium2, 8 NeuronCores.

Math (reference):
    img_n = l2norm(img);  mol_n = l2norm(mol)   [N, C], N=8192, C=128
    sim00 = img_n @ img_n.T / T ; sim01 = img_n @ mol_n.T / T ; sim11 = mol_n @ mol_n.T / T
    loss = mean(-diag(sim01))
         + 0.5 * mean( lse_row(sim00\\diag) + lse_row(sim01\\diag)
                     + lse_col(sim01\\diag) + lse_row(sim11\\diag) )

Device strategy (row-data-parallel over 8 cores):
  Each core gets the full img/mol (the "all-gathered" side) plus its own
  N/8 row slice.  On device: l2-normalize (fp32 stats, rsqrt via
  exp(-0.5*ln) so the scalar engine stays on one activation-table set),
  cast bf16, transpose via PE so both matmul operands have C on
  partitions; compute its [N/8 x N] blocks of the three gram matrices on
  PE; exp(10*x) fused with row-sum accumulation on the scalar engine
  straight out of PSUM; column-sums of exp(sim01) via ones-matmuls
  packed into PE column-groups and accumulated in PSUM across the row
  tiles (these give the sim10 row-sums).  Device outputs are O(N)
  partial sums; the host adds partials across cores, subtracts
  exp(diag), takes log and means (O(N) scalar work).

  Emission is pipelined: all input DMAs first; img normalize/transpose;
  all of sim00 (its exp stream hides mol's normalize/transpose); sim01;
  sim11.
"""

import numpy as np

import concourse.bass as bass
import concourse.tile as tile
from concourse import bacc, mybir
from concourse.bass_utils import run_bass_kernel_spmd
from concourse.masks import make_identity

F32 = mybir.dt.float32
BF16 = mybir.dt.bfloat16
AF = mybir.ActivationFunctionType
ALU = mybir.AluOpType

N_TOTAL = 8192
C = 128
N_CORES = 8
INV_T = 10.0  # 1 / TEMPERATURE


def _chunks(width, chunk):
    out = []
    s = 0
    while s < width:
        w = min(chunk, width - s)
        out.append((s, w))
        s += w
    return out


def build(n_total=N_TOTAL, n_cores=N_CORES, chunk=1536):
    """Build the per-core Tile program (SPMD: same program, different data)."""
    P = 128
    nm = n_total // n_cores          # rows owned by this core
    t_all = n_total // P             # 128-row tiles in the full matrices
    t_mine = nm // P                 # 128-row tiles in my slice
    chunk_list = _chunks(n_total, chunk)
    n_chunks = len(chunk_list)

    nc = bacc.Bacc("TRN2", target_bir_lowering=False, debug=False,
                   num_devices=n_cores)

    d_img = nc.dram_tensor("img", [n_total, C], F32, kind="ExternalInput").ap()
    d_mol = nc.dram_tensor("mol", [n_total, C], F32, kind="ExternalInput").ap()
    d_img_m = nc.dram_tensor("img_mine", [nm, C], F32, kind="ExternalInput").ap()
    d_mol_m = nc.dram_tensor("mol_mine", [nm, C], F32, kind="ExternalInput").ap()

    d_rowsums = nc.dram_tensor("rowsums", [P, 3 * t_mine * n_chunks], F32,
                               kind="ExternalOutput").ap()
    d_colsums = nc.dram_tensor("colsums", [1, n_total], F32,
                               kind="ExternalOutput").ap()
    d_diags = nc.dram_tensor("diags", [3, nm], F32, kind="ExternalOutput").ap()

    with tile.TileContext(nc) as tc:
        with (
            tc.tile_pool(name="big", bufs=1) as big,
            tc.tile_pool(name="work", bufs=3) as work,
            tc.tile_pool(name="expb", bufs=4) as expb,
            tc.tile_pool(name="sim", bufs=2, space="PSUM") as simp,
            tc.tile_pool(name="misc", bufs=2, space="PSUM") as miscp,
        ):
            ident = big.tile([P, P], BF16, tag="ident")
            make_identity(nc, ident)
            ones_b = big.tile([P, 1], BF16, tag="ones")
            nc.vector.memset(ones_b, 1.0)
            ones_f = big.tile([P, 1], F32, tag="ones_f")
            nc.vector.memset(ones_f, 1.0)

            specs = {
                "x": (d_img, t_all), "y": (d_mol, t_all),
                "xm": (d_img_m, t_mine), "ym": (d_mol_m, t_mine),
            }
            T = {}
            raw = {}
            for k, (_, tiles) in specs.items():
                T[k] = big.tile([P, tiles * P], BF16, tag=f"T_{k}",
                                name=f"T_{k}")
                raw[k] = big.tile([P, tiles, C], F32, tag=f"raw_{k}",
                                  name=f"raw_{k}")

            rows_sb = big.tile([P, 3 * t_mine * n_chunks], F32, tag="rows_sb")

            SLAB = 8

            def load(key):
                src, tiles = specs[key]
                src3 = src.rearrange("(t p) c -> p t c", p=P)
                for s in range(0, tiles, SLAB):
                    se = min(s + SLAB, tiles)
                    nc.sync.dma_start(out=raw[key][:, s:se, :],
                                      in_=src3[:, s:se, :])

            def norm_transpose(key):
                """sumsq -> rsqrt=exp(-ln/2) -> normalize(bf16) -> PE T"""
                _, tiles = specs[key]
                rw = raw[key]
                ss = big.tile([P, tiles], F32, tag=f"ss_{key}", name=f"ss_{key}")
                rs = big.tile([P, tiles], F32, tag=f"rs_{key}", name=f"rs_{key}")
                for s in range(0, tiles, SLAB):
                    se = min(s + SLAB, tiles)
                    sq = work.tile([P, SLAB, C], F32, tag="sq")
                    nc.vector.tensor_mul(sq[:, : se - s, :], rw[:, s:se, :],
                                         rw[:, s:se, :])
                    nc.vector.reduce_sum(out=ss[:, s:se], in_=sq[:, : se - s, :],
                                         axis=mybir.AxisListType.X)
                lg = work.tile([P, tiles], F32, tag="nrm_f")
                nc.scalar.activation(out=lg, in_=ss, func=AF.Ln)
                nc.scalar.activation(out=rs, in_=lg, func=AF.Exp, scale=-0.5)
                # normalize a slab at a time (rs broadcast along C, step-0 AP)
                for s in range(0, tiles, SLAB):
                    se = min(s + SLAB, tiles)
                    nrm = work.tile([P, SLAB, C], BF16, tag="nrm")
                    rs_sl = rs[:, s:se]
                    rs_b = bass.AP(tensor=rs_sl.tensor, offset=rs_sl.offset,
                                   ap=[rs_sl.ap[0], rs_sl.ap[1], [0, C]])
                    nc.vector.tensor_mul(nrm[:, : se - s, :], rw[:, s:se, :],
                                         rs_b)
                    for t in range(s, se):
                        grp = t % 4
                        if grp == 0:
                            pt = miscp.tile([P, 4 * P], BF16, tag="misc",
                                            name=f"pt_{key}_{t}")
                        nc.tensor.transpose(pt[:, grp * P:(grp + 1) * P],
                                            nrm[:, t - s, :], ident)
                        if grp == 3 or t == tiles - 1:
                            w = (grp + 1) * P
                            dst = T[key][:, (t - grp) * P:(t - grp) * P + w]
                            nc.vector.tensor_copy(out=dst, in_=pt[:, :w])

            def diag_block():
                # diag[i] = sum_c a_hat[c,i] * b_hat[c,i] -> ones-matmul
                for row, (a, b) in enumerate((("xm", "xm"), ("xm", "ym"),
                                              ("ym", "ym"))):
                    prod = work.tile([P, nm], F32, tag="diagprod", bufs=2)
                    nc.vector.tensor_mul(prod, T[a], T[b])
                    for s in range(0, nm, 512):
                        w = min(512, nm - s)
                        dp = miscp.tile([1, 512], F32, tag="misc")
                        nc.tensor.matmul(dp[:, :w], ones_f, prod[:, s:s + w],
                                         start=True, stop=True)
                        dsb = work.tile([1, 512], F32, tag="dsb")
                        nc.vector.tensor_copy(out=dsb[:, :w], in_=dp[:, :w])
                        nc.sync.dma_start(out=d_diags[row:row + 1, s:s + w],
                                          in_=dsb[:, :w])

            def gram_block(mat, a, b):
                """rows of T[a] x all of T[b]: exp rowsums (+colsums if mat==1).

                sim01 column sums: per 512-slice one ones-matmul per row
                tile, packed into PE column-group s (outputs land on PSUM
                partition 32*s), PSUM-accumulated across the 8 row tiles.
                """
                for ci, (cs, cw) in enumerate(chunk_list):
                    nsl = cw // 512
                    if mat == 1:
                        cp = miscp.tile([P, 512], F32, tag="misc",
                                        name=f"cp_{ci}")
                    for m in range(t_mine):
                        lhsT = T[a][:, m * P:(m + 1) * P]
                        ps = simp.tile([P, chunk], F32, tag="sim")
                        for s in range(0, cw, 512):
                            nc.tensor.matmul(ps[:, s:s + 512], lhsT,
                                             T[b][:, cs + s:cs + s + 512],
                                             start=True, stop=True)
                        eb = expb.tile([P, chunk], BF16, tag="eb",
                                       name=f"eb_{mat}_{ci}_{m}")
                        col = (mat * t_mine + m) * n_chunks + ci
                        nc.scalar.activation(out=eb[:, :cw], in_=ps[:, :cw],
                                             func=AF.Exp, scale=INV_T,
                                             accum_out=rows_sb[:, col:col + 1])
                        if mat == 1:
                            for s in range(nsl):
                                nc.tensor.matmul(
                                    cp[32 * s:32 * s + 1, :], ones_b,
                                    eb[:, s * 512:(s + 1) * 512],
                                    start=(m == 0), stop=(m == t_mine - 1),
                                    tile_position=(0, 32 * s),
                                    skip_group_check=True)
                    if mat == 1:
                        csb = work.tile([1, chunk], F32, tag="csb")
                        for s in range(nsl):
                            nc.vector.tensor_copy(
                                out=csb[0:1, s * 512:(s + 1) * 512],
                                in_=cp[32 * s:32 * s + 1, :])
                        nc.sync.dma_start(out=d_colsums[0:1, cs:cs + cw],
                                          in_=csb[0:1, :cw])

            # ---- pipelined emission order ----
            for k in ("x", "xm", "y", "ym"):
                load(k)
            norm_transpose("x")
            norm_transpose("xm")
            gram_block(0, "xm", "x")     # mol's transposes hide under this
            norm_transpose("y")
            norm_transpose("ym")
            diag_block()
            gram_block(1, "xm", "y")
            gram_block(2, "ym", "y")

            nc.sync.dma_start(out=d_rowsums, in_=rows_sb)

    nc.finalize()
    return nc


_NC_CACHE = {}


def _get_nc(n_total, n_cores, chunk=1536):
    key = (n_total, n_cores, chunk)
    if key not in _NC_CACHE:
        _NC_CACHE[key] = build(n_total, n_cores, chunk)
    return _NC_CACHE[key]


def _run(img, mol, trace=False, n_cores=N_CORES):
    img = np.ascontiguousarray(np.asarray(img, dtype=np.float32))
    mol = np.ascontiguousarray(np.asarray(mol, dtype=np.float32))
    n_total = img.shape[0]
    nm = n_total // n_cores
    nc = _get_nc(n_total, n_cores)

    in_maps = [
        {
            "img": img,
            "mol": mol,
            "img_mine": img[r * nm:(r + 1) * nm],
            "mol_mine": mol[r * nm:(r + 1) * nm],
        }
        for r in range(n_cores)
    ]
    res = run_bass_kernel_spmd(nc, in_maps, list(range(n_cores)), trace=trace)

    # ---------------- host combine: O(N) work ----------------
    P = 128
    t_mine = nm // P
    n_chunks = len(_chunks(n_total, 1536))
    rowsum = np.zeros((3, n_total), dtype=np.float64)
    colsum01 = np.zeros(n_total, dtype=np.float64)
    diags = np.zeros((3, n_total), dtype=np.float64)
    for r in range(n_cores):
        out = res.results[r]
        rw = out["rowsums"].astype(np.float64)      # [128, 3*t_mine*n_chunks]
        for mat in range(3):
            for m in range(t_mine):
                cols = slice((mat * t_mine + m) * n_chunks,
                             (mat * t_mine + m + 1) * n_chunks)
                rows = slice(r * nm + m * P, r * nm + (m + 1) * P)
                rowsum[mat, rows] = rw[:, cols].sum(axis=1)
        colsum01 += out["colsums"].astype(np.float64)[0]
        diags[:, r * nm:(r + 1) * nm] = out["diags"].astype(np.float64)

    ed = np.exp(INV_T * diags)  # exp(10 * raw cosine diag)
    s00 = rowsum[0] - ed[0]
    s01r = rowsum[1] - ed[1]
    s01c = colsum01 - ed[1]
    s11 = rowsum[2] - ed[2]
    pos = -INV_T * diags[1]
    loss = pos.mean() + 0.5 * (np.log(s00) + np.log(s01r)
                               + np.log(s01c) + np.log(s11)).mean()
    return np.array(loss, dtype=np.float32), res


def kernel(img_rep, mol_rep):
    loss, _ = _run(img_rep, mol_rep)
    return loss
